# revision 10
# baseline (speedup 1.0000x reference)
"""CoAttLayer Trainium2 kernel — pure data-parallel over batch on 8 NeuronCores.

Reference computation (per batch element b, T=1024, N=512, D=64, K=80):
  L  = tanh(R @ Wl @ P^T)                    (T, N)
  Hp = tanh(Wp @ P^T + (Wr @ R^T) @ L)       (K, N)
  Hr = tanh(Wr @ R^T + (Wp @ P^T) @ L^T)     (K, T)
  Ap = softmax(whp @ Hp), Ar = softmax(whr @ Hr)
  out[b] = concat(P^T @ Ap, R^T @ Ar)        (2D,)

Reassociated into D-sized contractions:
  Hp = [Wp | Wr] @ [P^T ; X]   with X = R^T @ L    (D, N)
  Hr = [Wr | Wp] @ [R^T ; Y]   with Y = P^T @ L^T  (D, T)

Design notes (validated against perfetto/NTFF traces):
 - All matmul operands bf16 (fp32 PSUM); rel err vs fp32 reference ~5.5e-3.
 - K<=64 matmuls stream moving rows at HALF rate; two K=64 matmuls packed
   into disjoint PE row groups via tile_position run fully concurrently.
   RlT and the L tiles are packed this way, using [Rt;Rt] / [Pt;Pt]
   replicas loaded into both partition halves (the replica halves are later
   overwritten by Y / X).
 - L^T is produced by the XBAR DMA-transpose engine (dma_start_transpose,
   16x128 tiles, ~14ns/tile) instead of 256 PE transposes: one whole-batch
   [128,4096]->[128,32,128] transpose per batch, triggered from the (idle)
   Sync sequencer. This removed ~30us of Tensor-engine busy time.
 - phase2(b) is emitted after phase1(b+1) so the DMA-transpose latency
   (~5us: DGE delay + 256 tiles + sem prop) is covered by a full phase of
   independent matmul work.
 - Softmax/pool tail: logits stay in their natural column layout [128,12]
   (12 N=1 matmuls), exp runs directly on that layout (no transposes, no
   max-subtraction — logits are provably small), and pooling matmuls use
   ones-AUGMENTED P/R (65th column) so the softmax denominators fall out of
   the same PSUM accumulation. One final 65x16 PE transpose + reciprocal +
   per-partition scale normalizes everything. This kills all smpool
   transposes and the half-clock serial tail.
 - Input loads are split across the Sync/Act HWDGE queues and the GpSimd
   SWDGE queue so batch-0/1 arrive early and no sequencer serializes >6
   triggers (each trigger costs ~0.7-1us on an in-order sequencer).
"""

import numpy as np

import concourse.bass as bass
import concourse.bacc as bacc
import concourse.mybir as mybir
import concourse.tile as tile
from concourse.bass_utils import run_bass_kernel_spmd

F32 = mybir.dt.float32
BF16 = mybir.dt.bfloat16
AF = mybir.ActivationFunctionType

B_LOC = 8      # batch elements per core
T, N, D, K = 1024, 512, 64, 80
TI = T // 128  # 8 t-tiles
NI = N // 128  # 4 n-tiles
DA = D + 1     # ones-augmented feature dim (col 64 = 1.0) for pooling sums
NCORES = 8


def build_kernel():
    nc = bacc.Bacc("TRN2", debug=False, target_bir_lowering=False)

    ins = {}
    for name, shape, dt in [
        ("review_bf", [B_LOC, T, DA], BF16),
        ("review_t", [B_LOC, D, T], BF16),
        ("post_bf", [B_LOC, N, DA], BF16),
        ("post_t", [B_LOC, D, N], BF16),
        ("wpack", [128, 226], BF16),
        ("ident", [128, 128], F32),
    ]:
        ins[name] = nc.declare_dram_parameter(name, shape, dt, isOutput=False)
    out_e = nc.declare_dram_parameter("out", [B_LOC, 2 * D], F32, isOutput=True)

    with tile.TileContext(nc) as tc:
        _body(nc, tc, ins, out_e)

    nc.compile()
    return nc


def _body(nc, tc, ins, out_e):
    from contextlib import ExitStack

    ctx = ExitStack()
    cpool = ctx.enter_context(tc.tile_pool(name="const", bufs=1))
    inpool = ctx.enter_context(tc.tile_pool(name="inputs", bufs=1))
    wk = ctx.enter_context(tc.tile_pool(name="work", bufs=2))
    ltp = ctx.enter_context(tc.tile_pool(name="ltw", bufs=3))
    ps_mm = ctx.enter_context(tc.tile_pool(name="ps_mm", bufs=2, space="PSUM"))
    ps_x = ctx.enter_context(tc.tile_pool(name="ps_x", bufs=1, space="PSUM"))
    ps_p2 = ctx.enter_context(tc.tile_pool(name="ps_p2", bufs=2, space="PSUM"))
    ps_small = ctx.enter_context(tc.tile_pool(name="ps_small", bufs=1, space="PSUM"))

    # ---- constants: one packed bf16 DMA (act queue) + fp32 identity ----
    wpack = cpool.tile([128, 226], BF16)
    nc.scalar.dma_start(out=wpack[:], in_=ins["wpack"].ap())
    wl2 = wpack[:, 0:64]
    wt_hp = wpack[:, 64:144]
    wt_hr = wpack[:, 144:224]
    whp_b = wpack[0:80, 224:225]
    whr_b = wpack[0:80, 225:226]
    ident_f = cpool.tile([128, 128], F32)

    # Persistent bf16 inputs (written once by merged DMAs, then read-only)
    r_ball = inpool.tile([128, B_LOC, TI, DA], BF16)
    p_ball = inpool.tile([128, B_LOC, NI, DA], BF16)
    hr_all = inpool.tile([128, B_LOC, T], BF16)
    hp_all = inpool.tile([128, B_LOC, N], BF16)

    rev_v = ins["review_bf"].ap().rearrange("b (p i) d -> p b i d", i=TI)
    post_v = ins["post_bf"].ap().rearrange("b (p j) d -> p b j d", j=NI)
    rt_v = ins["review_t"].ap().rearrange("b d t -> d b t")
    pt_v = ins["post_t"].ap().rearrange("b d t -> d b t")

    # batch 0 on sync (compute gates on it), batch 1 on act, rest on gpsimd
    def load_group(eng, lo, hi, balls=True):
        s = slice(lo, hi)
        for h in range(2):
            eng.dma_start(out=hr_all[h * D:(h + 1) * D, s, :], in_=rt_v[:, s])
            eng.dma_start(out=hp_all[h * D:(h + 1) * D, s, :], in_=pt_v[:, s])
        if balls:
            eng.dma_start(out=r_ball[:, s], in_=rev_v[:, s])
            eng.dma_start(out=p_ball[:, s], in_=post_v[:, s])

    load_group(nc.sync, 0, 1)
    load_group(nc.scalar, 1, 2, balls=False)
    nc.gpsimd.dma_start(out=r_ball[:, 1:2], in_=rev_v[:, 1:2])
    nc.gpsimd.dma_start(out=p_ball[:, 1:2], in_=post_v[:, 1:2])
    load_group(nc.gpsimd, 2, B_LOC)
    nc.sync.dma_start(out=ident_f[:], in_=ins["ident"].ap())

    st_all = [dict() for _ in range(B_LOC)]
    co_all = inpool.tile([DA, 2, B_LOC], F32)

    # 3-stage software pipeline, emitted one iteration at a time:
    #   stage A: phase1(b)      rlt/L/X matmuls + tanh evacs + XBAR triggers
    #   stage B: phase2(b-1)    hp/Y/hr matmuls (tanh of hr-half-1 deferred
    #                           to the iteration end so the ACT queue runs
    #                           the L-evacs early -> T1 trigger fires early)
    #   stage C: tail(b-2)      logits + exp + exp-weighted pooling
    # Interleaving order is hand-tuned so every cross-engine latency is
    # covered by dense independent matmul work from another stage.

    def p1_steps(b):
        st = st_all[b]
        st["hr_in"] = hr_all[:, b, :]
        st["hp_in"] = hp_all[:, b, :]
        rlt2 = wk.tile([128, N], BF16, tag="rlt2", name=f"rlt2{b}")
        l_sb = wk.tile([128, TI, N], BF16, tag="l_sb", name=f"l_sb{b}")
        st["ltw"] = ltp.tile([128, 2, 4, 4, 128], BF16, tag="ltw", name=f"ltw{b}")
        st["l_sb"] = l_sb
        lps = {}

        rps = ps_mm.tile([128, 512], F32, tag="mm", name=f"rlt_ps{b}")
        for h in range(2):
            rv = st["hr_in"][h * D:(h + 1) * D, :].rearrange(
                "p (c two k) -> p two c k", two=2, k=128
            )[:, h]
            # row-packed pair; col offset h*D lands half h on partitions h*D:
            nc.tensor.matmul(
                rps[h * D:(h + 1) * D, :], wl2[h * D:(h + 1) * D, :], rv,
                tile_position=(h * D, h * D), skip_group_check=True,
            )
        nc.vector.tensor_copy(rlt2[:], rps[:])
        yield

        def lp(p):
            t = ps_mm.tile([128, 2, N], F32, tag="mm", name=f"lps{b}_{p}")
            lps[p] = t
            for h in range(2):
                nc.tensor.matmul(
                    t[:, h],
                    rlt2[h * D:(h + 1) * D, p * 128:(p + 1) * 128],
                    st["hp_in"][h * D:(h + 1) * D, :],
                    tile_position=(h * D, 0),
                )

        def evac(p):
            nc.scalar.activation(l_sb[:, 2 * p:2 * p + 2, :], lps[p][:], AF.Tanh)

        lp(0)
        lp(1)
        evac(0)
        yield
        lp(2)
        evac(1)
        yield
        lp(3)
        evac(2)
        # t-half 0 tanh'd: XBAR transpose out[q, m, tp] = in[tp, m*128+q]
        nc.sync.dma_start_transpose(st["ltw"][:, 0], l_sb[:, 0:4, :])
        yield
        evac(3)
        nc.sync.dma_start_transpose(st["ltw"][:, 1], l_sb[:, 4:8, :])

    def x_steps(b):
        st = st_all[b]
        l_sb = st["l_sb"]
        xps = ps_x.tile([D, N], F32, tag="xps", name=f"xps{b}")

        def X(i):
            nc.tensor.matmul(
                xps[:], r_ball[:, b, i, 0:D], l_sb[:, i],
                start=(i == 0), stop=(i == TI - 1),
            )

        X(0)
        X(1)
        X(2)
        X(3)
        yield
        X(4)
        X(5)
        yield
        X(6)
        X(7)
        nc.vector.tensor_copy(st["hp_in"][D:128, :], xps[:])

    def p2m_steps(b):
        st = st_all[b]
        hp_bf = wk.tile([K, N], BF16, tag="hp_bf", name=f"hp_bf{b}")
        hr_bf = wk.tile([K, T], BF16, tag="hr_bf", name=f"hr_bf{b}")
        st["hp_bf"], st["hr_bf"] = hp_bf, hr_bf
        hps = ps_small.tile([K, N], F32, tag="small", name=f"hps{b}")
        nc.tensor.matmul(hps[:], wt_hp[:], st["hp_in"][:])
        st["hps"] = hps
        yield
        for c in range(2):
            yps = ps_p2.tile([D, 512], F32, tag="p2", name=f"yps{b}_{c}")
            for j in range(NI):
                nc.tensor.matmul(
                    yps[:], p_ball[:, b, j, 0:D], st["ltw"][:, c, :, j, :],
                    start=(j == 0), stop=(j == NI - 1),
                )
            nc.vector.tensor_copy(
                st["hr_in"][D:128, c * 512:(c + 1) * 512], yps[:]
            )
            hrs = ps_p2.tile([K, 512], F32, tag="p2", name=f"hrs{b}_{c}")
            nc.tensor.matmul(hrs[:], wt_hr[:], st["hr_in"][:, c * 512:(c + 1) * 512])
            st[f"hrs{c}"] = hrs
            if c == 0:
                yield

    def p2_tanh_mid(b):
        # emitted after evac0/evac1 so the L evacuations (which gate the
        # XBAR transposes and next iteration's X) clear the ACT queue first
        st = st_all[b]
        nc.scalar.activation(st["hp_bf"][:], st["hps"], AF.Tanh)
        nc.scalar.activation(st["hr_bf"][:, 0:512], st["hrs0"], AF.Tanh)

    def p2_tanh_tail(b):
        st = st_all[b]
        nc.scalar.activation(st["hr_bf"][:, 512:1024], st["hrs1"], AF.Tanh)

    def lgt_a(b):
        st = st_all[b]
        lg = ps_small.tile([128, 12], F32, tag="small", name=f"lg{b}")
        probs = wk.tile([128, 12], BF16, tag="probs", name=f"probs{b}")
        st["lg"], st["probs"] = lg, probs
        for j in range(NI):
            nc.tensor.matmul(
                lg[:, j:j + 1], st["hp_bf"][:, j * 128:(j + 1) * 128], whp_b,
                skip_group_check=True,
            )
        nc.scalar.activation(probs[:, 0:4], lg[:, 0:4], AF.Exp)

    def lgt_b(b):
        st = st_all[b]
        for i in range(TI):
            nc.tensor.matmul(
                st["lg"][:, 4 + i:5 + i], st["hr_bf"][:, i * 128:(i + 1) * 128],
                whr_b, skip_group_check=True,
            )
        nc.scalar.activation(st["probs"][:, 4:12], st["lg"][:, 4:12], AF.Exp)

    def pool(b):
        st = st_all[b]
        probs = st["probs"]
        co_ps = ps_small.tile([DA, 2], F32, tag="small", name=f"co_ps{b}")
        for j in range(NI):
            nc.tensor.matmul(
                co_ps[:, 0:1], p_ball[:, b, j, :], probs[:, j:j + 1],
                start=(j == 0), stop=(j == NI - 1), skip_group_check=True,
            )
        for i in range(TI):
            nc.tensor.matmul(
                co_ps[:, 1:2], r_ball[:, b, i, :], probs[:, 4 + i:5 + i],
                start=(i == 0), stop=(i == TI - 1), skip_group_check=True,
            )
        nc.vector.tensor_copy(co_all[:, :, b], co_ps[:])

    def step(g):
        if g is not None:
            next(g, None)

    # 4-deep pipeline: phase1(b) | X(b-1) | phase2(b-2) | logits/pool(b-3).
    # X consumes l_sb tanh'd LAST iteration (zero ACT wait) and its matmuls
    # cover the rlt->copy->lp and lp->evac latencies of phase1(b). phase2
    # consumes ltw two iterations after its XBAR transpose was triggered,
    # riding out the Sync queue's in-order issue latency (~2us per half).
    for it in range(B_LOC + 3):
        g1 = p1_steps(it) if it < B_LOC else None
        gx = x_steps(it - 1) if 1 <= it <= B_LOC else None
        g2 = p2m_steps(it - 2) if 2 <= it <= B_LOC + 1 else None
        tb = it - 3 if 0 <= it - 3 < B_LOC else None
        step(g1)            # rlt(b)
        step(g2)            # hp(b-2) (+inline tanh)
        step(gx)            # X0-3(b-1)
        step(g1)            # lp01 + evac0
        step(g2)            # Y0 + hr0 (+inline tanh)
        step(gx)            # X45(b-1)
        step(g1)            # lp2 + evac1
        if g2 is not None:
            p2_tanh_mid(it - 2)
        if tb is not None:
            lgt_a(tb)
        step(g2)            # Y1 + hr1 (tanh deferred)
        step(gx)            # X67(b-1) + writeback
        step(g1)            # lp3 + evac2 + T0
        if tb is not None:
            lgt_b(tb)
        step(g1)            # evac3 + T1
        if tb is not None:
            pool(tb)
        if g2 is not None:
            p2_tanh_tail(it - 2)

    # Transpose (65, 16) -> (16, 65); row h*8+b is the h-half of out[b],
    # col 64 the softmax denominator. Normalize with a per-partition scale.
    cot_ps = ps_small.tile([2 * B_LOC, DA], F32, tag="small")
    nc.tensor.transpose(
        cot_ps[:], co_all[:].rearrange("d h b -> d (h b)"), ident_f[0:DA, 0:DA]
    )
    rcp = inpool.tile([2 * B_LOC, 1], F32)
    nc.vector.reciprocal(rcp[:], cot_ps[:, D:DA])
    out_sb = inpool.tile([2 * B_LOC, D], F32)
    nc.vector.tensor_scalar_mul(out_sb[:], cot_ps[:, 0:D], rcp[:])
    nc.sync.dma_start(out=out_e.ap()[:, 0:D], in_=out_sb[0:B_LOC, :])
    nc.sync.dma_start(out=out_e.ap()[:, D:2 * D], in_=out_sb[B_LOC:2 * B_LOC, :])
    ctx.close()


_NC_CACHE = None


def _get_nc():
    global _NC_CACHE
    if _NC_CACHE is None:
        _NC_CACHE = build_kernel()
    return _NC_CACHE


def _prep_host_inputs(inputs):
    import ml_dtypes

    bf = ml_dtypes.bfloat16
    rev = np.ascontiguousarray(np.asarray(inputs["review_seq"], dtype=np.float32))
    post = np.ascontiguousarray(np.asarray(inputs["post_seq"], dtype=np.float32))
    wl = np.asarray(inputs["Wl"], dtype=np.float32)
    wr = np.asarray(inputs["Wr"], dtype=np.float32)
    wp = np.asarray(inputs["Wp"], dtype=np.float32)
    whr = np.asarray(inputs["whr"], dtype=np.float32)
    whp = np.asarray(inputs["whp"], dtype=np.float32)

    rev_bf = rev.astype(bf)
    post_bf = post.astype(bf)
    B = rev.shape[0]
    # ones-augmented copies for the pooling lhsT (col 64 = 1.0)
    rev_aug = np.concatenate(
        [rev_bf, np.ones((B, T, 1), dtype=bf)], axis=2
    )
    post_aug = np.concatenate(
        [post_bf, np.ones((B, N, 1), dtype=bf)], axis=2
    )
    # column order of the transposed layouts matches t = 8p + i / n = 4p + j:
    # rev_t[b, d, i*128 + p] = rev[b, 8p + i, d]
    rev_t = np.ascontiguousarray(
        rev_bf.reshape(B, 128, 8, 64).transpose(0, 3, 2, 1).reshape(B, 64, 1024)
    )
    post_t = np.ascontiguousarray(
        post_bf.reshape(B, 128, 4, 64).transpose(0, 3, 2, 1).reshape(B, 64, 512)
    )
    wpack = np.zeros((128, 226), dtype=bf)
    wpack[:, 0:64] = np.concatenate([wl, wl], axis=0).astype(bf)
    wpack[:, 64:144] = np.concatenate([wp.T, wr.T], axis=0).astype(bf)
    wpack[:, 144:224] = np.concatenate([wr.T, wp.T], axis=0).astype(bf)
    wpack[0:80, 224] = whp[0].astype(bf)
    wpack[0:80, 225] = whr[0].astype(bf)
    const = {
        "wpack": np.ascontiguousarray(wpack),
        "ident": np.eye(128, dtype=np.float32),
    }
    return rev_aug, rev_t, post_aug, post_t, const


def run_on_hw(inputs: dict, trace: bool = False, **kw):
    nc = _get_nc()
    rev_aug, rev_t, post_aug, post_t, const = _prep_host_inputs(inputs)
    in_maps = []
    for c in range(NCORES):
        s = slice(c * B_LOC, (c + 1) * B_LOC)
        m = {
            "review_bf": np.ascontiguousarray(rev_aug[s]),
            "review_t": np.ascontiguousarray(rev_t[s]),
            "post_bf": np.ascontiguousarray(post_aug[s]),
            "post_t": np.ascontiguousarray(post_t[s]),
        }
        m.update(const)
        in_maps.append(m)
    res = run_bass_kernel_spmd(nc, in_maps, list(range(NCORES)), trace=trace, **kw)
    out = np.concatenate([res.results[c]["out"] for c in range(NCORES)], axis=0)
    return out, res


def kernel(**inputs) -> np.ndarray:
    out, _ = run_on_hw(inputs, trace=False)
    return out.astype(np.float32)


# revision 11
# speedup vs baseline: 1.0185x; 1.0185x over previous
"""CoAttLayer Trainium2 kernel — pure data-parallel over batch on 8 NeuronCores.

Reference computation (per batch element b, T=1024, N=512, D=64, K=80):
  L  = tanh(R @ Wl @ P^T)                    (T, N)
  Hp = tanh(Wp @ P^T + (Wr @ R^T) @ L)       (K, N)
  Hr = tanh(Wr @ R^T + (Wp @ P^T) @ L^T)     (K, T)
  Ap = softmax(whp @ Hp), Ar = softmax(whr @ Hr)
  out[b] = concat(P^T @ Ap, R^T @ Ar)        (2D,)

Reassociated into D-sized contractions:
  Hp = [Wp | Wr] @ [P^T ; X]   with X = R^T @ L    (D, N)
  Hr = [Wr | Wp] @ [R^T ; Y]   with Y = P^T @ L^T  (D, T)

Design notes (validated against perfetto/NTFF traces):
 - All matmul operands bf16 (fp32 PSUM); rel err vs fp32 reference ~5.5e-3.
 - K<=64 matmuls stream moving rows at HALF rate; two K=64 matmuls packed
   into disjoint PE row groups via tile_position run fully concurrently.
   RlT and the L tiles are packed this way, using [Rt;Rt] / [Pt;Pt]
   replicas loaded into both partition halves (the replica halves are later
   overwritten by Y / X).
 - L^T is produced by the XBAR DMA-transpose engine (dma_start_transpose,
   16x128 tiles, ~14ns/tile) instead of 256 PE transposes: one whole-batch
   [128,4096]->[128,32,128] transpose per batch, triggered from the (idle)
   Sync sequencer. This removed ~30us of Tensor-engine busy time.
 - phase2(b) is emitted after phase1(b+1) so the DMA-transpose latency
   (~5us: DGE delay + 256 tiles + sem prop) is covered by a full phase of
   independent matmul work.
 - Softmax/pool tail: logits stay in their natural column layout [128,12]
   (12 N=1 matmuls), exp runs directly on that layout (no transposes, no
   max-subtraction — logits are provably small), and pooling matmuls use
   ones-AUGMENTED P/R (65th column) so the softmax denominators fall out of
   the same PSUM accumulation. One final 65x16 PE transpose + reciprocal +
   per-partition scale normalizes everything. This kills all smpool
   transposes and the half-clock serial tail.
 - Input loads are split across the Sync/Act HWDGE queues and the GpSimd
   SWDGE queue so batch-0/1 arrive early and no sequencer serializes >6
   triggers (each trigger costs ~0.7-1us on an in-order sequencer).
"""

import numpy as np

import concourse.bass as bass
import concourse.bacc as bacc
import concourse.mybir as mybir
import concourse.tile as tile
from concourse.bass_utils import run_bass_kernel_spmd

F32 = mybir.dt.float32
BF16 = mybir.dt.bfloat16
AF = mybir.ActivationFunctionType

B_LOC = 8      # batch elements per core
T, N, D, K = 1024, 512, 64, 80
TI = T // 128  # 8 t-tiles
NI = N // 128  # 4 n-tiles
DA = D + 1     # ones-augmented feature dim (col 64 = 1.0) for pooling sums
NCORES = 8


def build_kernel():
    nc = bacc.Bacc("TRN2", debug=False, target_bir_lowering=False)

    ins = {}
    for name, shape, dt in [
        ("review_bf", [B_LOC, T, DA], BF16),
        ("review_t", [B_LOC, D, T], BF16),
        ("post_bf", [B_LOC, N, DA], BF16),
        ("post_t", [B_LOC, D, N], BF16),
        ("wpack", [128, 226], BF16),
        ("ident", [128, 128], F32),
    ]:
        ins[name] = nc.declare_dram_parameter(name, shape, dt, isOutput=False)
    out_e = nc.declare_dram_parameter("out", [B_LOC, 2 * D], F32, isOutput=True)

    with tile.TileContext(nc) as tc:
        _body(nc, tc, ins, out_e)

    nc.compile()
    return nc


def _body(nc, tc, ins, out_e):
    from contextlib import ExitStack

    ctx = ExitStack()
    cpool = ctx.enter_context(tc.tile_pool(name="const", bufs=1))
    inpool = ctx.enter_context(tc.tile_pool(name="inputs", bufs=1))
    wk = ctx.enter_context(tc.tile_pool(name="work", bufs=2))
    ltp = ctx.enter_context(tc.tile_pool(name="ltw", bufs=3))
    ps_mm = ctx.enter_context(tc.tile_pool(name="ps_mm", bufs=2, space="PSUM"))
    ps_x = ctx.enter_context(tc.tile_pool(name="ps_x", bufs=1, space="PSUM"))
    ps_p2 = ctx.enter_context(tc.tile_pool(name="ps_p2", bufs=2, space="PSUM"))
    ps_small = ctx.enter_context(tc.tile_pool(name="ps_small", bufs=1, space="PSUM"))

    # ---- constants: one packed bf16 DMA (act queue) + fp32 identity ----
    wpack = cpool.tile([128, 226], BF16)
    nc.scalar.dma_start(out=wpack[:], in_=ins["wpack"].ap())
    wl2 = wpack[:, 0:64]
    wt_hp = wpack[:, 64:144]
    wt_hr = wpack[:, 144:224]
    whp_b = wpack[0:80, 224:225]
    whr_b = wpack[0:80, 225:226]
    ident_f = cpool.tile([128, 128], F32)

    # Persistent bf16 inputs (written once by merged DMAs, then read-only)
    r_ball = inpool.tile([128, B_LOC, TI, DA], BF16)
    p_ball = inpool.tile([128, B_LOC, NI, DA], BF16)
    hr_all = inpool.tile([128, B_LOC, T], BF16)
    hp_all = inpool.tile([128, B_LOC, N], BF16)

    rev_v = ins["review_bf"].ap().rearrange("b (p i) d -> p b i d", i=TI)
    post_v = ins["post_bf"].ap().rearrange("b (p j) d -> p b j d", j=NI)
    rt_v = ins["review_t"].ap().rearrange("b d t -> d b t")
    pt_v = ins["post_t"].ap().rearrange("b d t -> d b t")

    # batch 0 on sync (compute gates on it), batch 1 on act, rest on gpsimd
    def load_group(eng, lo, hi, balls=True):
        s = slice(lo, hi)
        for h in range(2):
            eng.dma_start(out=hr_all[h * D:(h + 1) * D, s, :], in_=rt_v[:, s])
            eng.dma_start(out=hp_all[h * D:(h + 1) * D, s, :], in_=pt_v[:, s])
        if balls:
            eng.dma_start(out=r_ball[:, s], in_=rev_v[:, s])
            eng.dma_start(out=p_ball[:, s], in_=post_v[:, s])

    load_group(nc.sync, 0, 1)
    load_group(nc.scalar, 1, 2, balls=False)
    nc.gpsimd.dma_start(out=r_ball[:, 1:2], in_=rev_v[:, 1:2])
    nc.gpsimd.dma_start(out=p_ball[:, 1:2], in_=post_v[:, 1:2])
    load_group(nc.gpsimd, 2, B_LOC)
    nc.sync.dma_start(out=ident_f[:], in_=ins["ident"].ap())

    st_all = [dict() for _ in range(B_LOC)]
    co_all = inpool.tile([DA, 2, B_LOC], F32)

    # 3-stage software pipeline, emitted one iteration at a time:
    #   stage A: phase1(b)      rlt/L/X matmuls + tanh evacs + XBAR triggers
    #   stage B: phase2(b-1)    hp/Y/hr matmuls (tanh of hr-half-1 deferred
    #                           to the iteration end so the ACT queue runs
    #                           the L-evacs early -> T1 trigger fires early)
    #   stage C: tail(b-2)      logits + exp + exp-weighted pooling
    # Interleaving order is hand-tuned so every cross-engine latency is
    # covered by dense independent matmul work from another stage.

    def p1_steps(b):
        st = st_all[b]
        st["hr_in"] = hr_all[:, b, :]
        st["hp_in"] = hp_all[:, b, :]
        rlt2 = wk.tile([128, N], BF16, tag="rlt2", name=f"rlt2{b}")
        l_sb = wk.tile([128, TI, N], BF16, tag="l_sb", name=f"l_sb{b}")
        st["ltw"] = ltp.tile([128, 2, 4, 4, 128], BF16, tag="ltw", name=f"ltw{b}")
        st["l_sb"] = l_sb
        lps = {}

        rps = ps_mm.tile([128, 512], F32, tag="mm", name=f"rlt_ps{b}")
        for h in range(2):
            rv = st["hr_in"][h * D:(h + 1) * D, :].rearrange(
                "p (c two k) -> p two c k", two=2, k=128
            )[:, h]
            # row-packed pair; col offset h*D lands half h on partitions h*D:
            nc.tensor.matmul(
                rps[h * D:(h + 1) * D, :], wl2[h * D:(h + 1) * D, :], rv,
                tile_position=(h * D, h * D), skip_group_check=True,
            )
        nc.vector.tensor_copy(rlt2[:], rps[:])
        yield

        def lp(p):
            t = ps_mm.tile([128, 2, N], F32, tag="mm", name=f"lps{b}_{p}")
            lps[p] = t
            for h in range(2):
                nc.tensor.matmul(
                    t[:, h],
                    rlt2[h * D:(h + 1) * D, p * 128:(p + 1) * 128],
                    st["hp_in"][h * D:(h + 1) * D, :],
                    tile_position=(h * D, 0),
                )

        def evac(p):
            nc.scalar.activation(l_sb[:, 2 * p:2 * p + 2, :], lps[p][:], AF.Tanh)

        lp(0)
        lp(1)
        evac(0)
        yield
        lp(2)
        evac(1)
        yield
        lp(3)
        evac(2)
        # t-half 0 tanh'd: XBAR transpose out[q, m, tp] = in[tp, m*128+q]
        nc.sync.dma_start_transpose(st["ltw"][:, 0], l_sb[:, 0:4, :])
        yield
        evac(3)
        nc.sync.dma_start_transpose(st["ltw"][:, 1], l_sb[:, 4:8, :])

    def x_steps(b):
        st = st_all[b]
        l_sb = st["l_sb"]
        xps = ps_x.tile([D, N], F32, tag="xps", name=f"xps{b}")

        def X(i):
            nc.tensor.matmul(
                xps[:], r_ball[:, b, i, 0:D], l_sb[:, i],
                start=(i == 0), stop=(i == TI - 1),
            )

        X(0)
        X(1)
        X(2)
        X(3)
        yield
        X(4)
        X(5)
        yield
        X(6)
        X(7)
        nc.vector.tensor_copy(st["hp_in"][D:128, :], xps[:])

    def p2m_steps(b):
        st = st_all[b]
        hp_bf = wk.tile([K, N], BF16, tag="hp_bf", name=f"hp_bf{b}")
        hr_bf = wk.tile([K, T], BF16, tag="hr_bf", name=f"hr_bf{b}")
        st["hp_bf"], st["hr_bf"] = hp_bf, hr_bf
        hps = ps_small.tile([K, N], F32, tag="small", name=f"hps{b}")
        nc.tensor.matmul(hps[:], wt_hp[:], st["hp_in"][:])
        st["hps"] = hps
        yield
        for c in range(2):
            yps = ps_p2.tile([D, 512], F32, tag="p2", name=f"yps{b}_{c}")
            for j in range(NI):
                nc.tensor.matmul(
                    yps[:], p_ball[:, b, j, 0:D], st["ltw"][:, c, :, j, :],
                    start=(j == 0), stop=(j == NI - 1),
                )
            nc.vector.tensor_copy(
                st["hr_in"][D:128, c * 512:(c + 1) * 512], yps[:]
            )
            hrs = ps_p2.tile([K, 512], F32, tag="p2", name=f"hrs{b}_{c}")
            nc.tensor.matmul(hrs[:], wt_hr[:], st["hr_in"][:, c * 512:(c + 1) * 512])
            st[f"hrs{c}"] = hrs
            if c == 0:
                yield

    def p2_tanh_mid(b):
        # emitted after evac0/evac1 so the L evacuations (which gate the
        # XBAR transposes and next iteration's X) clear the ACT queue first
        st = st_all[b]
        nc.scalar.activation(st["hp_bf"][:], st["hps"], AF.Tanh)
        nc.scalar.activation(st["hr_bf"][:, 0:512], st["hrs0"], AF.Tanh)

    def p2_tanh_tail(b):
        st = st_all[b]
        nc.scalar.activation(st["hr_bf"][:, 512:1024], st["hrs1"], AF.Tanh)

    def lgt_a(b):
        st = st_all[b]
        lg = ps_small.tile([128, 12], F32, tag="small", name=f"lg{b}")
        probs = wk.tile([128, 12], BF16, tag="probs", name=f"probs{b}")
        st["lg"], st["probs"] = lg, probs
        for j in range(NI):
            nc.tensor.matmul(
                lg[:, j:j + 1], st["hp_bf"][:, j * 128:(j + 1) * 128], whp_b,
                skip_group_check=True,
            )
        nc.scalar.activation(probs[:, 0:4], lg[:, 0:4], AF.Exp)

    def lgt_b(b):
        st = st_all[b]
        for i in range(TI):
            nc.tensor.matmul(
                st["lg"][:, 4 + i:5 + i], st["hr_bf"][:, i * 128:(i + 1) * 128],
                whr_b, skip_group_check=True,
            )
        nc.scalar.activation(st["probs"][:, 4:12], st["lg"][:, 4:12], AF.Exp)

    def pool(b):
        st = st_all[b]
        probs = st["probs"]
        co_ps = ps_small.tile([DA, 2], F32, tag="small", name=f"co_ps{b}")
        for j in range(NI):
            nc.tensor.matmul(
                co_ps[:, 0:1], p_ball[:, b, j, :], probs[:, j:j + 1],
                start=(j == 0), stop=(j == NI - 1), skip_group_check=True,
            )
        for i in range(TI):
            nc.tensor.matmul(
                co_ps[:, 1:2], r_ball[:, b, i, :], probs[:, 4 + i:5 + i],
                start=(i == 0), stop=(i == TI - 1), skip_group_check=True,
            )
        nc.vector.tensor_copy(co_all[:, :, b], co_ps[:])

    def step(g):
        if g is not None:
            next(g, None)

    # 4-deep pipeline: phase1(b) | X(b-1) | phase2(b-2) | logits/pool(b-3).
    # X consumes l_sb tanh'd LAST iteration (zero ACT wait) and its matmuls
    # cover the rlt->copy->lp and lp->evac latencies of phase1(b). phase2
    # consumes ltw two iterations after its XBAR transpose was triggered,
    # riding out the Sync queue's in-order issue latency (~2us per half).
    for it in range(B_LOC + 2):
        g1 = p1_steps(it) if it < B_LOC else None
        gx = x_steps(it - 1) if 1 <= it <= B_LOC else None
        g2 = p2m_steps(it - 2) if 2 <= it <= B_LOC + 1 else None
        tb = it - 3 if 0 <= it - 3 < B_LOC else None
        step(g1)            # rlt(b)
        step(g2)            # hp(b-2) (+inline tanh)
        step(gx)            # X0-3(b-1)
        step(g1)            # lp01 + evac0
        step(g2)            # Y0 + hr0 (+inline tanh)
        step(gx)            # X45(b-1)
        step(g1)            # lp2 + evac1
        if g2 is not None:
            p2_tanh_mid(it - 2)
        if tb is not None:
            lgt_a(tb)
        step(g2)            # Y1 + hr1 (tanh deferred)
        step(gx)            # X67(b-1) + writeback
        step(g1)            # lp3 + evac2 + T0
        if tb is not None:
            lgt_b(tb)
        step(g1)            # evac3 + T1
        if tb is not None:
            pool(tb)
        if g2 is not None:
            p2_tanh_tail(it - 2)
        if it == B_LOC + 1:
            # drain compaction: batch 7's logits/pool follow immediately,
            # interleaved with batch 6's, keeping the PE stream dense so the
            # clock governor stays at full rate through the tail.
            lgt_a(B_LOC - 1)
            lgt_b(B_LOC - 1)
            pool(B_LOC - 1)

    # Transpose (65, 16) -> (16, 65); row h*8+b is the h-half of out[b],
    # col 64 the softmax denominator. Normalize with a per-partition scale.
    cot_ps = ps_small.tile([2 * B_LOC, DA], F32, tag="small")
    nc.tensor.transpose(
        cot_ps[:], co_all[:].rearrange("d h b -> d (h b)"), ident_f[0:DA, 0:DA]
    )
    rcp = inpool.tile([2 * B_LOC, 1], F32)
    nc.vector.reciprocal(rcp[:], cot_ps[:, D:DA])
    out_sb = inpool.tile([2 * B_LOC, D], F32)
    nc.vector.tensor_scalar_mul(out_sb[:], cot_ps[:, 0:D], rcp[:])
    nc.sync.dma_start(out=out_e.ap()[:, 0:D], in_=out_sb[0:B_LOC, :])
    nc.sync.dma_start(out=out_e.ap()[:, D:2 * D], in_=out_sb[B_LOC:2 * B_LOC, :])
    ctx.close()


_NC_CACHE = None


def _get_nc():
    global _NC_CACHE
    if _NC_CACHE is None:
        _NC_CACHE = build_kernel()
    return _NC_CACHE


def _prep_host_inputs(inputs):
    import ml_dtypes

    bf = ml_dtypes.bfloat16
    rev = np.ascontiguousarray(np.asarray(inputs["review_seq"], dtype=np.float32))
    post = np.ascontiguousarray(np.asarray(inputs["post_seq"], dtype=np.float32))
    wl = np.asarray(inputs["Wl"], dtype=np.float32)
    wr = np.asarray(inputs["Wr"], dtype=np.float32)
    wp = np.asarray(inputs["Wp"], dtype=np.float32)
    whr = np.asarray(inputs["whr"], dtype=np.float32)
    whp = np.asarray(inputs["whp"], dtype=np.float32)

    rev_bf = rev.astype(bf)
    post_bf = post.astype(bf)
    B = rev.shape[0]
    # ones-augmented copies for the pooling lhsT (col 64 = 1.0)
    rev_aug = np.concatenate(
        [rev_bf, np.ones((B, T, 1), dtype=bf)], axis=2
    )
    post_aug = np.concatenate(
        [post_bf, np.ones((B, N, 1), dtype=bf)], axis=2
    )
    # column order of the transposed layouts matches t = 8p + i / n = 4p + j:
    # rev_t[b, d, i*128 + p] = rev[b, 8p + i, d]
    rev_t = np.ascontiguousarray(
        rev_bf.reshape(B, 128, 8, 64).transpose(0, 3, 2, 1).reshape(B, 64, 1024)
    )
    post_t = np.ascontiguousarray(
        post_bf.reshape(B, 128, 4, 64).transpose(0, 3, 2, 1).reshape(B, 64, 512)
    )
    wpack = np.zeros((128, 226), dtype=bf)
    wpack[:, 0:64] = np.concatenate([wl, wl], axis=0).astype(bf)
    wpack[:, 64:144] = np.concatenate([wp.T, wr.T], axis=0).astype(bf)
    wpack[:, 144:224] = np.concatenate([wr.T, wp.T], axis=0).astype(bf)
    wpack[0:80, 224] = whp[0].astype(bf)
    wpack[0:80, 225] = whr[0].astype(bf)
    const = {
        "wpack": np.ascontiguousarray(wpack),
        "ident": np.eye(128, dtype=np.float32),
    }
    return rev_aug, rev_t, post_aug, post_t, const


def run_on_hw(inputs: dict, trace: bool = False, **kw):
    nc = _get_nc()
    rev_aug, rev_t, post_aug, post_t, const = _prep_host_inputs(inputs)
    in_maps = []
    for c in range(NCORES):
        s = slice(c * B_LOC, (c + 1) * B_LOC)
        m = {
            "review_bf": np.ascontiguousarray(rev_aug[s]),
            "review_t": np.ascontiguousarray(rev_t[s]),
            "post_bf": np.ascontiguousarray(post_aug[s]),
            "post_t": np.ascontiguousarray(post_t[s]),
        }
        m.update(const)
        in_maps.append(m)
    res = run_bass_kernel_spmd(nc, in_maps, list(range(NCORES)), trace=trace, **kw)
    out = np.concatenate([res.results[c]["out"] for c in range(NCORES)], axis=0)
    return out, res


def kernel(**inputs) -> np.ndarray:
    out, _ = run_on_hw(inputs, trace=False)
    return out.astype(np.float32)
